# revision 35
# baseline (speedup 1.0000x reference)
"""GCN ActorCriticNet on 8 Trainium2 NeuronCores (Bass/Tile SPMD).

reference:
    h = relu(segment_sum(h[src], dst) @ W + b)   x3 layers
    mN = mean(h); v = mN@Wv+bv; pi = concat(h@Wpg+bpg, mN@Wpd+bpd)

Sharding: nodes (and in-edges, partitioned by dst) across 8 cores. Node
feature tables are replicated per core in a permuted layout (core-major,
in-degree-rank order, 16384 rows/core so the 131072-row table splits into
exactly four 32768-row ranges for int16 gather indices); after each layer
an AllGather rebuilds the table on every core.

Per core each layer, h[src] rows (256B: 64 bf16 feats + 64 pad) are
bulk-gathered with `dma_gather` (~2.3ns/row on 4 SWDGE queues - the Q7
descriptor-generation rate is the fundamental bound for data-dependent
access, measured ~110GB/s/core). The edge stream is sorted by (src-range,
dst-window); segment-sum runs on the tensor engine as selector matmuls:
per 128-edge tile a one-hot [edge, node-rank] bf16 matrix (DVE is_equal
of dst-rank vs iota, batched per gather to amortize DVE op overhead)
accumulates into the window's PSUM bank slot. The slot schedule is shared
by all 8 cores (max-padded per (range,window), min 128) so one SPMD
program serves all cores. Then aggT via PE transpose (+ones row) @ [W;b]
-> ReLU. Layer 3 computes pi = h@Wpg per node (DVE) and per-core node
sums; host applies the tiny heads (bpg, Wpd/bpd, Wv/bv) while unsharding.
"""
import os
import hashlib
import numpy as np

_N = 100000
_E = 1000000
_D = 64
_ROW = 128                       # table row width (64 feats + 64 pad), bf16
_NC = 8
_P = 128
_NPC = _N // _NC                 # 12500 nodes per core
_WIN = (_NPC + _P - 1) // _P     # 98 windows per core
_WROWS = _WIN * _P               # 12544 rows written per core
_RPC = 16384                     # table rows per core (padded for ranges)
_TAB = _NC * _RPC                # 131072 table rows
_RB = [0, 32768, 65536, 98304, 131072]
_NR = 4
_GRP = 8                         # windows per psum group
_NGRP = (_WIN + _GRP - 1) // _GRP
_QRK = 4096                      # ranks per core per table quarter
_QWIN = _QRK // _P               # windows per quarter = 32
_QGRP = _QWIN // _GRP            # groups per quarter = 4
_PAD_REL = 300.0

_cache = {}


def _preprocess(src, dst):
    src = np.asarray(src, np.int64)
    dst = np.asarray(dst, np.int64)
    deg = np.bincount(dst, minlength=_N)

    degc = deg.reshape(_NC, _NPC)
    order = np.argsort(degc, axis=1, kind="stable")          # rank -> local id
    ranks = np.empty((_NC, _NPC), np.int64)
    np.put_along_axis(ranks, order,
                      np.broadcast_to(np.arange(_NPC), (_NC, _NPC)).copy(), axis=1)
    # quarter-segment table layout: rank segment q of every core occupies
    # table rows [q*32768, (q+1)*32768) (rank-major within the segment), so
    # each int16 gather range == one quarter and the per-quarter AllGather
    # writes a contiguous slice as soon as its 32 windows are done.
    cidx = np.arange(_NC)[:, None]
    qseg = ranks // _QRK
    posmap = (qseg * 32768 + cidx * _QRK + ranks % _QRK).reshape(-1)

    srcpos = posmap[src]                      # table row of each edge's src
    rng_e = srcpos >> 15                      # range id per edge
    core_e = dst // _NPC
    rank_e = ranks[core_e, dst % _NPC]        # dst rank within its core
    win_e = rank_e >> 7                       # window id

    key_crw = (core_e * _NR + rng_e) * _WIN + win_e
    counts = np.bincount(key_crw, minlength=_NC * _NR * _WIN).reshape(
        _NC, _NR, _WIN)
    # shared schedule: per (r, w) run >= 128 so a tile spans <= 2 windows
    Q = np.maximum(counts.max(axis=0), 128)   # [NR, WIN]

    NI = np.zeros((_NR, _NGRP), np.int64)
    for g in range(_NGRP):
        w0, w1 = g * _GRP, min((g + 1) * _GRP, _WIN)
        for r in range(_NR):
            s = int(Q[r, w0:w1].sum())
            NI[r, g] = ((s + 127) // 128) * 128

    tile_base = {}
    ntiles = 0
    for g in range(_NGRP):
        for r in range(_NR):
            tile_base[(r, g)] = ntiles
            ntiles += int(NI[r, g]) // 128

    offw = np.zeros((_NR, _WIN), np.int64)
    for r in range(_NR):
        for g in range(_NGRP):
            w0, w1 = g * _GRP, min((g + 1) * _GRP, _WIN)
            off = 0
            for w in range(w0, w1):
                offw[r, w] = off
                off += int(Q[r, w])

    eorder = np.lexsort((rank_e, win_e, rng_e, core_e))
    k2 = key_crw[eorder]
    run_start = np.zeros(_NC * _NR * _WIN + 1, np.int64)
    np.cumsum(np.bincount(k2, minlength=_NC * _NR * _WIN), out=run_start[1:])
    pos_in_run = np.arange(_E) - run_start[k2]

    idx_streams = np.zeros((_NC, ntiles * 128), np.int16)     # pad -> row 0
    rel_streams = np.full((_NC, ntiles * 128), _PAD_REL, np.float32)
    e_c = core_e[eorder]
    e_r = rng_e[eorder]
    e_w = win_e[eorder]
    e_rank = rank_e[eorder]
    e_srcpos = srcpos[eorder]
    e_g = e_w // _GRP
    tb = np.array([[tile_base[(r, g)] for g in range(_NGRP)]
                   for r in range(_NR)], np.int64)
    spos = tb[e_r, e_g] * 128 + offw[e_r, e_w] + pos_in_run
    idx_streams[e_c, spos] = (e_srcpos - np.array(_RB, np.int64)[e_r]
                              ).astype(np.int16)

    tile_w0 = np.zeros(ntiles, np.int64)
    for g in range(_NGRP):
        w0, w1 = g * _GRP, min((g + 1) * _GRP, _WIN)
        for r in range(_NR):
            base = tile_base[(r, g)]
            nt = int(NI[r, g]) // 128
            cur = w0
            for t in range(nt):
                lo = t * 128
                while cur < w1 - 1 and offw[r, cur] + Q[r, cur] <= lo:
                    cur += 1
                tile_w0[base + t] = cur
    tile_of = spos // 128
    rel_streams[e_c, spos] = (e_rank - 128 * tile_w0[tile_of]
                              ).astype(np.float32)
    assert rel_streams[e_c, spos].max() < 256.0 and \
        rel_streams[e_c, spos].min() >= 0

    # rank bounds per (tile, window) over all cores
    tw_key = tile_of * _WIN + e_w
    o2 = np.argsort(tw_key, kind="stable")
    tk = tw_key[o2]
    rk = e_rank[o2] - (e_w[o2] * 128)
    uniq, start_u = np.unique(tk, return_index=True)
    end_u = np.append(start_u[1:], len(tk))
    rmin = {}
    rmax = {}
    for u, s, e in zip(uniq, start_u, end_u):
        rmin[int(u)] = int(rk[s:e].min())
        rmax[int(u)] = int(rk[s:e].max())

    passes = [[] for _ in range(_WIN)]
    for g in range(_NGRP):
        w0g, w1g = g * _GRP, min((g + 1) * _GRP, _WIN)
        for w in range(w0g, w1g):
            plist = []
            for r in range(_NR):
                base = tile_base[(r, g)]
                a = offw[r, w]
                b = a + int(Q[r, w])
                t0, t1 = int(a // 128), int((b - 1) // 128)
                for t in range(t0, t1 + 1):
                    gt = base + t
                    u = gt * _WIN + w
                    if u not in rmin:
                        continue
                    # PE psum base partition: 0 or 64; width 64 or 128
                    if rmin[u] >= 64:
                        r0, r1 = 64, 128
                    elif rmax[u] < 64:
                        r0, r1 = 0, 64
                    else:
                        r0, r1 = 0, 128
                    ib = 128 * (w - int(tile_w0[gt]))
                    assert ib in (0, 128), (w, tile_w0[gt])
                    plist.append((gt, r0, r1, ib))
            if not plist:
                gt = tile_base[(0, g)] + int(offw[0, w] // 128)
                ib = 128 * (w - int(tile_w0[gt]))
                assert ib in (0, 128)
                plist.append((gt, 0, 128, ib))
            gt, r0, r1, ib = plist[0]
            plist[0] = (gt, 0, 128, ib)       # first pass covers full window
            passes[w] = plist

    sched = dict(Q=Q, NI=NI, tile_base=tile_base, ntiles=ntiles,
                 passes=passes, tile_w0=tile_w0)
    return dict(order=order, posmap=posmap, sched=sched,
                idx=idx_streams, rel=rel_streams)


def _build(sched, dbg=False):
    from contextlib import ExitStack
    import concourse.bacc as bacc
    import concourse.tile as tile
    from concourse import mybir
    from concourse.masks import make_identity

    NI = sched["NI"]
    tile_base = sched["tile_base"]
    ntiles = sched["ntiles"]
    passes = sched["passes"]
    f32 = mybir.dt.float32
    bf16 = mybir.dt.bfloat16
    i16 = mybir.dt.int16

    tloc = {}
    for (r, g), base in tile_base.items():
        for t in range(int(NI[r, g]) // 128):
            tloc[base + t] = (r, g, t)

    nc = bacc.Bacc("TRN2", target_bir_lowering=False, debug=False,
                   num_devices=_NC, num_swdge_queues=4)
    xt = nc.dram_tensor("xt", [_TAB, _ROW], bf16, kind="ExternalInput").ap()
    idx = nc.dram_tensor("idx", [_P, ntiles * 8], i16, kind="ExternalInput").ap()
    rel = nc.dram_tensor("rel", [_P, ntiles], bf16, kind="ExternalInput").ap()
    iota = nc.dram_tensor("iota", [_P, 256], bf16, kind="ExternalInput").ap()
    w_aps = [nc.dram_tensor(f"w{i}", [_D + 1, _D], bf16, kind="ExternalInput").ap()
             for i in (1, 2, 3)]
    wpg = nc.dram_tensor("wpg", [_P, _D], f32, kind="ExternalInput").ap()
    pi_out = nc.dram_tensor("pi", [_WROWS, 1], f32, kind="ExternalOutput").ap()
    mp_out = nc.dram_tensor("mp", [1, _D], f32, kind="ExternalOutput").ap()
    h1t = nc.dram_tensor("h1t", [_TAB, _ROW], bf16, addr_space="Shared").ap()
    h2t = nc.dram_tensor("h2t", [_TAB, _ROW], bf16, addr_space="Shared").ap()
    hloc = [[nc.dram_tensor(f"hloc{i}q{q}", [_QRK, _ROW], bf16).ap()
             for q in range(4)] for i in (1, 2)]
    dbg_aps = {}
    if dbg:
        for i in (1, 2, 3):
            dbg_aps[i] = nc.dram_tensor(f"dbg{i}", [_WROWS, _D], bf16,
                                        kind="ExternalOutput").ap()

    with tile.TileContext(nc) as tc, ExitStack() as ctx:
        const = ctx.enter_context(tc.tile_pool(name="const", bufs=1))
        gpool = ctx.enter_context(tc.tile_pool(name="g", bufs=3))
        sbpool = ctx.enter_context(tc.tile_pool(name="selb", bufs=2))
        spool = ctx.enter_context(tc.tile_pool(name="s", bufs=4))
        hpool = ctx.enter_context(tc.tile_pool(name="h", bufs=4))
        ppool = ctx.enter_context(tc.tile_pool(name="psum", bufs=2, space="PSUM"))

        ident = const.tile([_P, _P], bf16)
        make_identity(nc, ident[:])
        idx_sb = const.tile([_P, ntiles * 8], i16)
        nc.sync.dma_start(idx_sb[:], idx[:])
        rel_sb = const.tile([_P, ntiles], bf16)
        nc.sync.dma_start(rel_sb[:], rel[:])
        iota_sb = const.tile([_P, 256], bf16)
        nc.sync.dma_start(iota_sb[:], iota[:])
        w_sb = []
        for i, w_ap in enumerate(w_aps):
            t = const.tile([_D + 1, _D], bf16, tag=f"w{i}")
            nc.sync.dma_start(t[:], w_ap[:])
            w_sb.append(t)
        wpg_sb = const.tile([_P, _D], f32)
        nc.sync.dma_start(wpg_sb[:], wpg[:])
        n_aggT = 3
        aggT = []
        for i in range(n_aggT):
            t = const.tile([_D + 1, _P], bf16, tag=f"aggT{i}")
            nc.vector.memset(t[_D:_D + 1, :], 1.0)
            aggT.append(t)
        acc = const.tile([_P, _D], f32)
        nc.vector.memset(acc[:], 0.0)
        ones_col = const.tile([_P, 1], f32)
        nc.vector.memset(ones_col[:], 1.0)

        def emit_chain(l, w, pwt, hstage, pistage, slot):
            agg_sb = spool.tile([_P, _D], bf16, tag="agg")
            nc.scalar.copy(agg_sb[:], pwt[:])
            psum_t = ppool.tile([_D, _P], bf16, tag="pt")
            nc.tensor.transpose(out=psum_t[:], in_=agg_sb[:],
                                identity=ident[:])
            at = aggT[w % n_aggT]
            nc.scalar.copy(at[:_D, :], psum_t[:])
            psum_h = ppool.tile([_P, _D], f32, tag="ph")
            nc.tensor.matmul(out=psum_h[:], lhsT=at[:], rhs=w_sb[l][:],
                             start=True, stop=True)
            hb = hstage[:, slot * _D:(slot + 1) * _D]
            nc.scalar.activation(hb, psum_h[:],
                                 mybir.ActivationFunctionType.Relu)
            if dbg:
                nc.sync.dma_start(
                    dbg_aps[l + 1][w * _P:(w + 1) * _P, :], hb)
            if l == 2:
                nvalid = min(_NPC - w * _P, _P)
                tmp = spool.tile([_P, _D], f32, tag="tmp")
                nc.vector.tensor_tensor(out=tmp[:], in0=hb,
                                        in1=wpg_sb[:],
                                        op=mybir.AluOpType.mult)
                nc.vector.tensor_reduce(pistage[:, slot:slot + 1], tmp[:],
                                        axis=mybir.AxisListType.X,
                                        op=mybir.AluOpType.add)
                nc.vector.tensor_tensor(out=acc[:nvalid, :],
                                        in0=acc[:nvalid, :],
                                        in1=hb[:nvalid],
                                        op=mybir.AluOpType.add)

        SELC = 8                      # tiles per selector build
        tabs = [xt, h1t, h2t]
        qctr = 0
        for l in range(3):
            src_t = tabs[l]
            for g in range(_NGRP):
                gbufs = {}
                for r in range(_NR):
                    ni = int(NI[r, g])
                    tb = tile_base[(r, g)]
                    gb = gpool.tile([_P, (ni // 128) * _ROW], bf16,
                                    tag=f"g{r}")
                    g3 = gb[:].rearrange("p (s d) -> p s d", d=_ROW)
                    nc.gpsimd.dma_gather(
                        g3, src_t[_RB[r]:_RB[r + 1], :],
                        idx_sb[:, tb * 8: tb * 8 + ni // 16],
                        ni, ni, _ROW, elem_step=_ROW,
                        single_packet=False, queue_num=qctr % 4)
                    qctr += 1
                    gbufs[r] = gb
                sel_chunks = {}

                def get_sel(r_, ck):
                    if (r_, ck) in sel_chunks:
                        return sel_chunks[(r_, ck)]
                    nt = int(NI[r_, g]) // 128
                    tb = tile_base[(r_, g)] + ck * SELC
                    n2 = min(SELC, nt - ck * SELC)
                    sel = sbpool.tile([_P, SELC * 256], bf16, tag=f"sb{r_}")
                    nc.vector.tensor_tensor(
                        out=sel[:, :n2 * 256].rearrange(
                            "p (t i) -> p t i", i=256),
                        in0=rel_sb[:, tb:tb + n2]
                            .rearrange("p (t o) -> p t o", o=1)
                            .to_broadcast([_P, n2, 256]),
                        in1=iota_sb[:]
                            .rearrange("p (o i) -> p o i", o=1)
                            .to_broadcast([_P, n2, 256]),
                        op=mybir.AluOpType.is_equal)
                    sel_chunks[(r_, ck)] = sel
                    return sel

                w0g, w1g = g * _GRP, min((g + 1) * _GRP, _WIN)
                ng = w1g - w0g
                hstage = hpool.tile([_P, _GRP * _D], bf16, tag="hstage")
                pistage = None
                if l == 2:
                    pistage = hpool.tile([_P, _GRP], f32, tag="pistage")
                for w in range(w0g, w1g):
                    pwt = ppool.tile([_P, _D], f32, tag="pw")
                    plist = passes[w]
                    for k, (gt, r0, r1, ib) in enumerate(plist):
                        r_, g_, tcol = tloc[gt]
                        sel = get_sel(r_, tcol // SELC)
                        tc_ = tcol % SELC
                        nc.tensor.matmul(
                            out=pwt[r0:r1, :],
                            lhsT=sel[:, tc_ * 256 + ib + r0:
                                     tc_ * 256 + ib + r1],
                            rhs=gbufs[r_][:, tcol * _ROW:tcol * _ROW + _D],
                            start=(k == 0), stop=(k == len(plist) - 1),
                            skip_group_check=True)
                    emit_chain(l, w, pwt, hstage, pistage, w - w0g)
                # batched group stores
                if l < 2:
                    q = min(g // _QGRP, 3)
                    r0_ = (w0g - q * _QWIN) * _P
                    dst = hloc[l][q][r0_:r0_ + ng * _P, :_D]
                    nc.sync.dma_start(
                        dst.rearrange("(t p) d -> p t d", p=_P),
                        hstage[:, :ng * _D].rearrange(
                            "p (t d) -> p t d", d=_D))
                    # per-quarter AllGather as soon as its windows are done
                    if g in (_QGRP - 1, 2 * _QGRP - 1, 3 * _QGRP - 1,
                             _NGRP - 1):
                        q = min(g // _QGRP, 3)
                        nc.gpsimd.collective_compute(
                            "AllGather", mybir.AluOpType.bypass,
                            replica_groups=[list(range(_NC))],
                            ins=[hloc[l][q][:]],
                            outs=[tabs[l + 1][q * 32768:(q + 1) * 32768, :]])
                else:
                    dst = pi_out[w0g * _P:w0g * _P + ng * _P, :]
                    nc.sync.dma_start(
                        dst.rearrange("(t p) o -> p t o", p=_P),
                        pistage[:, :ng].rearrange("p (t o) -> p t o", o=1))

        psum_m = ppool.tile([1, _D], f32, tag="pm")
        nc.tensor.matmul(out=psum_m[:], lhsT=ones_col[:], rhs=acc[:],
                         start=True, stop=True)
        msb = spool.tile([1, _D], f32, tag="msb")
        nc.vector.tensor_copy(msb[:], psum_m[:])
        nc.sync.dma_start(mp_out[:], msb[:])

    nc.compile()
    return nc


def _install_ntff_shim():
    """Make bass_utils trace=True work under axon when antenv.axon_hooks is
    missing (builds the hook from trn_agent_boot's ctypes factory)."""
    import sys
    import types
    try:
        from antenv.axon_hooks import get_axon_ntff_profile_hook  # noqa: F401
        return
    except ImportError:
        pass
    try:
        from trn_agent_boot.trn_boot import _ntff_profile_via_ctypes
        hook = _ntff_profile_via_ctypes('/opt/axon/libaxon_pjrt.so')
    except Exception:
        return
    mod = types.ModuleType('antenv.axon_hooks')
    mod.get_axon_ntff_profile_hook = lambda: hook
    mod.set_axon_ntff_profile_hook = lambda h: None
    sys.modules['antenv.axon_hooks'] = mod
    import antenv
    antenv.axon_hooks = mod


def _wrap_idx(idx_stream, sched):
    """Per-core int16 stream -> [128, ntiles*8] wrapped dma_gather layout."""
    NI = sched["NI"]
    tile_base = sched["tile_base"]
    out = np.zeros((_P, sched["ntiles"] * 8), np.int16)
    for g in range(_NGRP):
        for r in range(_NR):
            ni = int(NI[r, g])
            tb = tile_base[(r, g)]
            chunk = idx_stream[tb * 128: tb * 128 + ni]
            w16 = chunk.reshape(ni // 16, 16).T
            out[:, tb * 8: tb * 8 + ni // 16] = np.tile(w16, (8, 1))
    return out


def kernel(x, src, dst, W1, b1, W2, b2, W3, b3, Wpg, bpg, Wpd, bpd, Wv, bv):
    import ml_dtypes
    from concourse.bass_utils import run_bass_kernel_spmd
    bf16 = ml_dtypes.bfloat16

    x = np.asarray(x, np.float32)
    src_i = np.asarray(src)
    dst_i = np.asarray(dst)

    key = hashlib.sha256(src_i.tobytes() + dst_i.tobytes()).hexdigest()
    if key not in _cache:
        pre = _preprocess(src_i, dst_i)
        nc = _build(pre["sched"])
        idx_w = np.stack([_wrap_idx(pre["idx"][c], pre["sched"])
                          for c in range(_NC)])
        rel_w = np.stack([pre["rel"][c].reshape(-1, 128).T.astype(bf16)
                          for c in range(_NC)])
        _cache.clear()
        _cache[key] = (pre, nc, idx_w, rel_w)
    pre, nc, idx_w, rel_w = _cache[key]
    order = pre["order"]

    xt = np.zeros((_TAB, _ROW), bf16)
    xt[pre["posmap"], :_D] = x.astype(bf16)

    def w65(W, b):
        return np.concatenate([np.asarray(W, np.float32),
                               np.asarray(b, np.float32).reshape(1, _D)],
                              axis=0).astype(bf16)

    wpg_rep = np.tile(np.asarray(Wpg, np.float32).reshape(1, _D), (_P, 1))
    iota_t = np.tile(np.arange(256, dtype=np.float32).astype(bf16), (_P, 1))

    base = {"xt": xt, "w1": w65(W1, b1), "w2": w65(W2, b2), "w3": w65(W3, b3),
            "wpg": wpg_rep, "iota": iota_t}
    in_maps = [dict(base, idx=idx_w[c], rel=rel_w[c]) for c in range(_NC)]

    trace = bool(int(os.environ.get("GCN_TRACE", "0")))
    if trace:
        _install_ntff_shim()
    res = run_bass_kernel_spmd(nc, in_maps, list(range(_NC)), trace=trace)
    if trace and res.exec_time_ns is not None:
        print(f"HW exec time: {res.exec_time_ns} ns")

    pi = np.empty(_N + 1, np.float32)
    msum = np.zeros(_D, np.float64)
    for c in range(_NC):
        pi[c * _NPC + order[c]] = res.results[c]["pi"][:_NPC, 0]
        msum += res.results[c]["mp"][0].astype(np.float64)
    pi[:_N] += np.float32(np.asarray(bpg).reshape(()))
    mN = (msum / _N).astype(np.float32).reshape(1, _D)
    pi[_N] = (mN @ np.asarray(Wpd, np.float32)
              + np.asarray(bpd, np.float32)).reshape(())
    v = (mN @ np.asarray(Wv, np.float32) + np.asarray(bv, np.float32)).reshape(1, 1)
    return pi.reshape(_N + 1, 1), v


# revision 39
# speedup vs baseline: 1.1859x; 1.1859x over previous
"""GCN ActorCriticNet on 8 Trainium2 NeuronCores (Bass/Tile SPMD).

reference:
    h = relu(segment_sum(h[src], dst) @ W + b)   x3 layers
    mN = mean(h); v = mN@Wv+bv; pi = concat(h@Wpg+bpg, mN@Wpd+bpd)

Sharding: nodes (and in-edges, partitioned by dst) across 8 cores. Node
feature tables are replicated per core in a permuted layout (core-major,
in-degree-rank order, 16384 rows/core so the 131072-row table splits into
exactly four 32768-row ranges for int16 gather indices); after each layer
an AllGather rebuilds the table on every core.

Per core each layer, h[src] rows (256B: 64 bf16 feats + 64 pad) are
bulk-gathered with `dma_gather` (~2.3ns/row on 4 SWDGE queues - the Q7
descriptor-generation rate is the fundamental bound for data-dependent
access, measured ~110GB/s/core). The edge stream is sorted by (src-range,
dst-window); segment-sum runs on the tensor engine as selector matmuls:
per 128-edge tile a one-hot [edge, node-rank] bf16 matrix (DVE is_equal
of dst-rank vs iota, batched per gather to amortize DVE op overhead)
accumulates into the window's PSUM bank slot. The slot schedule is shared
by all 8 cores (max-padded per (range,window), min 128) so one SPMD
program serves all cores. Then aggT via PE transpose (+ones row) @ [W;b]
-> ReLU. Layer 3 computes pi = h@Wpg per node (DVE) and per-core node
sums; host applies the tiny heads (bpg, Wpd/bpd, Wv/bv) while unsharding.
"""
import os
import hashlib
import numpy as np

_N = 100000
_E = 1000000
_D = 64
_ROW = 128                       # table row width (64 feats + 64 pad), bf16
_NC = 8
_P = 128
_NPC = _N // _NC                 # 12500 nodes per core
_WIN = (_NPC + _P - 1) // _P     # 98 windows per core
_WROWS = _WIN * _P               # 12544 rows written per core
_RPC = 16384                     # table rows per core (padded for ranges)
_TAB = _NC * _RPC                # 131072 table rows
_RB = [0, 32768, 65536, 98304, 131072]
_NR = 4
_GRP = 8                         # windows per psum group
_NGRP = (_WIN + _GRP - 1) // _GRP
_NSEG = 2                        # table segments (pipelined AllGathers)
_SRK = _RPC // _NSEG             # ranks per core per segment
_SWIN = _SRK // _P               # windows per segment
_SGRP = _SWIN // _GRP            # groups per segment
_PAD_REL = 300.0

_cache = {}


def _preprocess(src, dst):
    src = np.asarray(src, np.int64)
    dst = np.asarray(dst, np.int64)
    deg = np.bincount(dst, minlength=_N)

    degc = deg.reshape(_NC, _NPC)
    order = np.argsort(degc, axis=1, kind="stable")          # rank -> local id
    ranks = np.empty((_NC, _NPC), np.int64)
    np.put_along_axis(ranks, order,
                      np.broadcast_to(np.arange(_NPC), (_NC, _NPC)).copy(), axis=1)
    # segmented table layout: rank segment s of every core occupies a
    # contiguous rank-major table slice, so each segment's AllGather can
    # fire as soon as its windows are done (pipelined with later compute).
    cidx = np.arange(_NC)[:, None]
    sseg = ranks // _SRK
    posmap = (sseg * (_NC * _SRK) + cidx * _SRK + ranks % _SRK).reshape(-1)

    srcpos = posmap[src]                      # table row of each edge's src
    rng_e = srcpos >> 15                      # range id per edge
    core_e = dst // _NPC
    rank_e = ranks[core_e, dst % _NPC]        # dst rank within its core
    win_e = rank_e >> 7                       # window id

    key_crw = (core_e * _NR + rng_e) * _WIN + win_e
    counts = np.bincount(key_crw, minlength=_NC * _NR * _WIN).reshape(
        _NC, _NR, _WIN)
    # shared schedule: per (r, w) run >= 128 so a tile spans <= 2 windows
    Q = np.maximum(counts.max(axis=0), 128)   # [NR, WIN]

    NI = np.zeros((_NR, _NGRP), np.int64)
    for g in range(_NGRP):
        w0, w1 = g * _GRP, min((g + 1) * _GRP, _WIN)
        for r in range(_NR):
            s = int(Q[r, w0:w1].sum())
            NI[r, g] = ((s + 127) // 128) * 128

    tile_base = {}
    ntiles = 0
    for g in range(_NGRP):
        for r in range(_NR):
            tile_base[(r, g)] = ntiles
            ntiles += int(NI[r, g]) // 128

    offw = np.zeros((_NR, _WIN), np.int64)
    for r in range(_NR):
        for g in range(_NGRP):
            w0, w1 = g * _GRP, min((g + 1) * _GRP, _WIN)
            off = 0
            for w in range(w0, w1):
                offw[r, w] = off
                off += int(Q[r, w])

    eorder = np.lexsort((rank_e, win_e, rng_e, core_e))
    k2 = key_crw[eorder]
    run_start = np.zeros(_NC * _NR * _WIN + 1, np.int64)
    np.cumsum(np.bincount(k2, minlength=_NC * _NR * _WIN), out=run_start[1:])
    pos_in_run = np.arange(_E) - run_start[k2]

    idx_streams = np.zeros((_NC, ntiles * 128), np.int16)     # pad -> row 0
    rel_streams = np.full((_NC, ntiles * 128), _PAD_REL, np.float32)
    e_c = core_e[eorder]
    e_r = rng_e[eorder]
    e_w = win_e[eorder]
    e_rank = rank_e[eorder]
    e_srcpos = srcpos[eorder]
    e_g = e_w // _GRP
    tb = np.array([[tile_base[(r, g)] for g in range(_NGRP)]
                   for r in range(_NR)], np.int64)
    spos = tb[e_r, e_g] * 128 + offw[e_r, e_w] + pos_in_run
    idx_streams[e_c, spos] = (e_srcpos - np.array(_RB, np.int64)[e_r]
                              ).astype(np.int16)

    tile_w0 = np.zeros(ntiles, np.int64)
    for g in range(_NGRP):
        w0, w1 = g * _GRP, min((g + 1) * _GRP, _WIN)
        for r in range(_NR):
            base = tile_base[(r, g)]
            nt = int(NI[r, g]) // 128
            cur = w0
            for t in range(nt):
                lo = t * 128
                while cur < w1 - 1 and offw[r, cur] + Q[r, cur] <= lo:
                    cur += 1
                tile_w0[base + t] = cur
    tile_of = spos // 128
    rel_streams[e_c, spos] = (e_rank - 128 * tile_w0[tile_of]
                              ).astype(np.float32)
    assert rel_streams[e_c, spos].max() < 256.0 and \
        rel_streams[e_c, spos].min() >= 0

    # rank bounds per (tile, window) over all cores
    tw_key = tile_of * _WIN + e_w
    o2 = np.argsort(tw_key, kind="stable")
    tk = tw_key[o2]
    rk = e_rank[o2] - (e_w[o2] * 128)
    uniq, start_u = np.unique(tk, return_index=True)
    end_u = np.append(start_u[1:], len(tk))
    rmin = {}
    rmax = {}
    for u, s, e in zip(uniq, start_u, end_u):
        rmin[int(u)] = int(rk[s:e].min())
        rmax[int(u)] = int(rk[s:e].max())

    passes = [[] for _ in range(_WIN)]
    for g in range(_NGRP):
        w0g, w1g = g * _GRP, min((g + 1) * _GRP, _WIN)
        for w in range(w0g, w1g):
            plist = []
            for r in range(_NR):
                base = tile_base[(r, g)]
                a = offw[r, w]
                b = a + int(Q[r, w])
                t0, t1 = int(a // 128), int((b - 1) // 128)
                for t in range(t0, t1 + 1):
                    gt = base + t
                    u = gt * _WIN + w
                    if u not in rmin:
                        continue
                    # PE psum base partition: 0 or 64; width 64 or 128
                    if rmin[u] >= 64:
                        r0, r1 = 64, 128
                    elif rmax[u] < 64:
                        r0, r1 = 0, 64
                    else:
                        r0, r1 = 0, 128
                    ib = 128 * (w - int(tile_w0[gt]))
                    assert ib in (0, 128), (w, tile_w0[gt])
                    plist.append((gt, r0, r1, ib))
            if not plist:
                gt = tile_base[(0, g)] + int(offw[0, w] // 128)
                ib = 128 * (w - int(tile_w0[gt]))
                assert ib in (0, 128)
                plist.append((gt, 0, 128, ib))
            gt, r0, r1, ib = plist[0]
            plist[0] = (gt, 0, 128, ib)       # first pass covers full window
            passes[w] = plist

    sched = dict(Q=Q, NI=NI, tile_base=tile_base, ntiles=ntiles,
                 passes=passes, tile_w0=tile_w0)
    return dict(order=order, posmap=posmap, sched=sched,
                idx=idx_streams, rel=rel_streams)


def _build(sched, dbg=False):
    from contextlib import ExitStack
    import concourse.bacc as bacc
    import concourse.tile as tile
    from concourse import mybir
    from concourse.masks import make_identity

    NI = sched["NI"]
    tile_base = sched["tile_base"]
    ntiles = sched["ntiles"]
    passes = sched["passes"]
    f32 = mybir.dt.float32
    bf16 = mybir.dt.bfloat16
    i16 = mybir.dt.int16

    tloc = {}
    for (r, g), base in tile_base.items():
        for t in range(int(NI[r, g]) // 128):
            tloc[base + t] = (r, g, t)

    nc = bacc.Bacc("TRN2", target_bir_lowering=False, debug=False,
                   num_devices=_NC, num_swdge_queues=4)
    xt = nc.dram_tensor("xt", [_TAB, _ROW], bf16, kind="ExternalInput").ap()
    idx = nc.dram_tensor("idx", [_P, ntiles * 8], i16, kind="ExternalInput").ap()
    rel = nc.dram_tensor("rel", [_P, ntiles], bf16, kind="ExternalInput").ap()
    iota = nc.dram_tensor("iota", [_P, 256], bf16, kind="ExternalInput").ap()
    w_aps = [nc.dram_tensor(f"w{i}", [_D + 1, _D], bf16, kind="ExternalInput").ap()
             for i in (1, 2, 3)]
    wpg = nc.dram_tensor("wpg", [_P, _D], f32, kind="ExternalInput").ap()
    pi_out = nc.dram_tensor("pi", [_WROWS, 1], f32, kind="ExternalOutput").ap()
    mp_out = nc.dram_tensor("mp", [1, _D], f32, kind="ExternalOutput").ap()
    h1t = nc.dram_tensor("h1t", [_TAB, _ROW], bf16, addr_space="Shared").ap()
    h2t = nc.dram_tensor("h2t", [_TAB, _ROW], bf16, addr_space="Shared").ap()
    hloc = [[nc.dram_tensor(f"hloc{i}s{q}", [_SRK, _ROW], bf16).ap()
             for q in range(_NSEG)] for i in (1, 2)]
    dbg_aps = {}
    if dbg:
        for i in (1, 2, 3):
            dbg_aps[i] = nc.dram_tensor(f"dbg{i}", [_WROWS, _D], bf16,
                                        kind="ExternalOutput").ap()

    with tile.TileContext(nc) as tc, ExitStack() as ctx:
        const = ctx.enter_context(tc.tile_pool(name="const", bufs=1))
        gpool = ctx.enter_context(tc.tile_pool(name="g", bufs=3))
        sbpool = ctx.enter_context(tc.tile_pool(name="selb", bufs=2))
        spool = ctx.enter_context(tc.tile_pool(name="s", bufs=4))
        hpool = ctx.enter_context(tc.tile_pool(name="h", bufs=4))
        ppool = ctx.enter_context(tc.tile_pool(name="psum", bufs=2, space="PSUM"))

        ident = const.tile([_P, _P], bf16)
        make_identity(nc, ident[:])
        idx_sb = const.tile([_P, ntiles * 8], i16)
        nc.sync.dma_start(idx_sb[:], idx[:])
        rel_sb = const.tile([_P, ntiles], bf16)
        nc.sync.dma_start(rel_sb[:], rel[:])
        iota_sb = const.tile([_P, 256], bf16)
        nc.sync.dma_start(iota_sb[:], iota[:])
        w_sb = []
        for i, w_ap in enumerate(w_aps):
            t = const.tile([_D + 1, _D], bf16, tag=f"w{i}")
            nc.sync.dma_start(t[:], w_ap[:])
            w_sb.append(t)
        wpg_sb = const.tile([_P, _D], f32)
        nc.sync.dma_start(wpg_sb[:], wpg[:])
        n_aggT = 3
        aggT = []
        for i in range(n_aggT):
            t = const.tile([_D + 1, _P], bf16, tag=f"aggT{i}")
            nc.vector.memset(t[_D:_D + 1, :], 1.0)
            aggT.append(t)
        acc = const.tile([_P, _D], f32)
        nc.vector.memset(acc[:], 0.0)
        ones_col = const.tile([_P, 1], f32)
        nc.vector.memset(ones_col[:], 1.0)

        def emit_chain(l, w, pwt, hstage, pistage, slot):
            agg_sb = spool.tile([_P, _D], bf16, tag="agg")
            nc.scalar.copy(agg_sb[:], pwt[:])
            psum_t = ppool.tile([_D, _P], bf16, tag="pt")
            nc.tensor.transpose(out=psum_t[:], in_=agg_sb[:],
                                identity=ident[:])
            at = aggT[w % n_aggT]
            nc.scalar.copy(at[:_D, :], psum_t[:])
            psum_h = ppool.tile([_P, _D], f32, tag="ph")
            nc.tensor.matmul(out=psum_h[:], lhsT=at[:], rhs=w_sb[l][:],
                             start=True, stop=True)
            hb = hstage[:, slot * _D:(slot + 1) * _D]
            nc.scalar.activation(hb, psum_h[:],
                                 mybir.ActivationFunctionType.Relu)
            if dbg:
                nc.sync.dma_start(
                    dbg_aps[l + 1][w * _P:(w + 1) * _P, :], hb)
            if l == 2:
                nvalid = min(_NPC - w * _P, _P)
                tmp = spool.tile([_P, _D], f32, tag="tmp")
                nc.vector.tensor_tensor(out=tmp[:], in0=hb,
                                        in1=wpg_sb[:],
                                        op=mybir.AluOpType.mult)
                nc.vector.tensor_reduce(pistage[:, slot:slot + 1], tmp[:],
                                        axis=mybir.AxisListType.X,
                                        op=mybir.AluOpType.add)
                nc.vector.tensor_tensor(out=acc[:nvalid, :],
                                        in0=acc[:nvalid, :],
                                        in1=hb[:nvalid],
                                        op=mybir.AluOpType.add)

        SELC = 8                      # tiles per selector build
        tabs = [xt, h1t, h2t]
        qctr = 0
        for l in range(3):
            src_t = tabs[l]
            for g in range(_NGRP):
                gbufs = {}
                for r in range(_NR):
                    ni = int(NI[r, g])
                    tb = tile_base[(r, g)]
                    gb = gpool.tile([_P, (ni // 128) * _ROW], bf16,
                                    tag=f"g{r}")
                    g3 = gb[:].rearrange("p (s d) -> p s d", d=_ROW)
                    nc.gpsimd.dma_gather(
                        g3, src_t[_RB[r]:_RB[r + 1], :],
                        idx_sb[:, tb * 8: tb * 8 + ni // 16],
                        ni, ni, _ROW, elem_step=_ROW,
                        single_packet=False, queue_num=qctr % 4)
                    qctr += 1
                    gbufs[r] = gb
                sel_chunks = {}

                def get_sel(r_, ck):
                    if (r_, ck) in sel_chunks:
                        return sel_chunks[(r_, ck)]
                    nt = int(NI[r_, g]) // 128
                    tb = tile_base[(r_, g)] + ck * SELC
                    n2 = min(SELC, nt - ck * SELC)
                    sel = sbpool.tile([_P, SELC * 256], bf16, tag=f"sb{r_}")
                    nc.vector.tensor_tensor(
                        out=sel[:, :n2 * 256].rearrange(
                            "p (t i) -> p t i", i=256),
                        in0=rel_sb[:, tb:tb + n2]
                            .rearrange("p (t o) -> p t o", o=1)
                            .to_broadcast([_P, n2, 256]),
                        in1=iota_sb[:]
                            .rearrange("p (o i) -> p o i", o=1)
                            .to_broadcast([_P, n2, 256]),
                        op=mybir.AluOpType.is_equal)
                    sel_chunks[(r_, ck)] = sel
                    return sel

                w0g, w1g = g * _GRP, min((g + 1) * _GRP, _WIN)
                ng = w1g - w0g
                hstage = hpool.tile([_P, _GRP * _D], bf16, tag="hstage")
                pistage = None
                if l == 2:
                    pistage = hpool.tile([_P, _GRP], f32, tag="pistage")
                for w in range(w0g, w1g):
                    pwt = ppool.tile([_P, _D], f32, tag="pw")
                    plist = passes[w]
                    for k, (gt, r0, r1, ib) in enumerate(plist):
                        r_, g_, tcol = tloc[gt]
                        sel = get_sel(r_, tcol // SELC)
                        tc_ = tcol % SELC
                        nc.tensor.matmul(
                            out=pwt[r0:r1, :],
                            lhsT=sel[:, tc_ * 256 + ib + r0:
                                     tc_ * 256 + ib + r1],
                            rhs=gbufs[r_][:, tcol * _ROW:tcol * _ROW + _D],
                            start=(k == 0), stop=(k == len(plist) - 1),
                            skip_group_check=True)
                    emit_chain(l, w, pwt, hstage, pistage, w - w0g)
                # batched group stores
                if l < 2:
                    q = min(g // _SGRP, _NSEG - 1)
                    r0_ = (w0g - q * _SWIN) * _P
                    dst = hloc[l][q][r0_:r0_ + ng * _P, :_D]
                    nc.sync.dma_start(
                        dst.rearrange("(t p) d -> p t d", p=_P),
                        hstage[:, :ng * _D].rearrange(
                            "p (t d) -> p t d", d=_D))
                    # per-segment AllGather as soon as its windows are done
                    bounds = [s * _SGRP - 1 for s in range(1, _NSEG)] + \
                        [_NGRP - 1]
                    if g in bounds:
                        q = min(g // _SGRP, _NSEG - 1)
                        rows = _NC * _SRK
                        nc.gpsimd.collective_compute(
                            "AllGather", mybir.AluOpType.bypass,
                            replica_groups=[list(range(_NC))],
                            ins=[hloc[l][q][:]],
                            outs=[tabs[l + 1][q * rows:(q + 1) * rows, :]])
                else:
                    dst = pi_out[w0g * _P:w0g * _P + ng * _P, :]
                    nc.sync.dma_start(
                        dst.rearrange("(t p) o -> p t o", p=_P),
                        pistage[:, :ng].rearrange("p (t o) -> p t o", o=1))

        psum_m = ppool.tile([1, _D], f32, tag="pm")
        nc.tensor.matmul(out=psum_m[:], lhsT=ones_col[:], rhs=acc[:],
                         start=True, stop=True)
        msb = spool.tile([1, _D], f32, tag="msb")
        nc.vector.tensor_copy(msb[:], psum_m[:])
        nc.sync.dma_start(mp_out[:], msb[:])

    nc.compile()
    return nc


def _install_ntff_shim():
    """Make bass_utils trace=True work under axon when antenv.axon_hooks is
    missing (builds the hook from trn_agent_boot's ctypes factory)."""
    import sys
    import types
    try:
        from antenv.axon_hooks import get_axon_ntff_profile_hook  # noqa: F401
        return
    except ImportError:
        pass
    try:
        from trn_agent_boot.trn_boot import _ntff_profile_via_ctypes
        hook = _ntff_profile_via_ctypes('/opt/axon/libaxon_pjrt.so')
    except Exception:
        return
    mod = types.ModuleType('antenv.axon_hooks')
    mod.get_axon_ntff_profile_hook = lambda: hook
    mod.set_axon_ntff_profile_hook = lambda h: None
    sys.modules['antenv.axon_hooks'] = mod
    import antenv
    antenv.axon_hooks = mod


def _wrap_idx(idx_stream, sched):
    """Per-core int16 stream -> [128, ntiles*8] wrapped dma_gather layout."""
    NI = sched["NI"]
    tile_base = sched["tile_base"]
    out = np.zeros((_P, sched["ntiles"] * 8), np.int16)
    for g in range(_NGRP):
        for r in range(_NR):
            ni = int(NI[r, g])
            tb = tile_base[(r, g)]
            chunk = idx_stream[tb * 128: tb * 128 + ni]
            w16 = chunk.reshape(ni // 16, 16).T
            out[:, tb * 8: tb * 8 + ni // 16] = np.tile(w16, (8, 1))
    return out


def kernel(x, src, dst, W1, b1, W2, b2, W3, b3, Wpg, bpg, Wpd, bpd, Wv, bv):
    import ml_dtypes
    from concourse.bass_utils import run_bass_kernel_spmd
    bf16 = ml_dtypes.bfloat16

    x = np.asarray(x, np.float32)
    src_i = np.asarray(src)
    dst_i = np.asarray(dst)

    key = hashlib.sha256(src_i.tobytes() + dst_i.tobytes()).hexdigest()
    if key not in _cache:
        pre = _preprocess(src_i, dst_i)
        nc = _build(pre["sched"])
        idx_w = np.stack([_wrap_idx(pre["idx"][c], pre["sched"])
                          for c in range(_NC)])
        rel_w = np.stack([pre["rel"][c].reshape(-1, 128).T.astype(bf16)
                          for c in range(_NC)])
        _cache.clear()
        _cache[key] = (pre, nc, idx_w, rel_w)
    pre, nc, idx_w, rel_w = _cache[key]
    order = pre["order"]

    xt = np.zeros((_TAB, _ROW), bf16)
    xt[pre["posmap"], :_D] = x.astype(bf16)

    def w65(W, b):
        return np.concatenate([np.asarray(W, np.float32),
                               np.asarray(b, np.float32).reshape(1, _D)],
                              axis=0).astype(bf16)

    wpg_rep = np.tile(np.asarray(Wpg, np.float32).reshape(1, _D), (_P, 1))
    iota_t = np.tile(np.arange(256, dtype=np.float32).astype(bf16), (_P, 1))

    base = {"xt": xt, "w1": w65(W1, b1), "w2": w65(W2, b2), "w3": w65(W3, b3),
            "wpg": wpg_rep, "iota": iota_t}
    in_maps = [dict(base, idx=idx_w[c], rel=rel_w[c]) for c in range(_NC)]

    trace = bool(int(os.environ.get("GCN_TRACE", "0")))
    if trace:
        _install_ntff_shim()
    res = run_bass_kernel_spmd(nc, in_maps, list(range(_NC)), trace=trace)
    if trace and res.exec_time_ns is not None:
        print(f"HW exec time: {res.exec_time_ns} ns")

    pi = np.empty(_N + 1, np.float32)
    msum = np.zeros(_D, np.float64)
    for c in range(_NC):
        pi[c * _NPC + order[c]] = res.results[c]["pi"][:_NPC, 0]
        msum += res.results[c]["mp"][0].astype(np.float64)
    pi[:_N] += np.float32(np.asarray(bpg).reshape(()))
    mN = (msum / _N).astype(np.float32).reshape(1, _D)
    pi[_N] = (mN @ np.asarray(Wpd, np.float32)
              + np.asarray(bpd, np.float32)).reshape(())
    v = (mN @ np.asarray(Wv, np.float32) + np.asarray(bv, np.float32)).reshape(1, 1)
    return pi.reshape(_N + 1, 1), v
